# revision 1
# baseline (speedup 1.0000x reference)
"""Trainium2 Bass kernel for a 2-layer GCN discriminator (GCNConv -> sigmoid) x2.

Strategy
--------
With N=4096 nodes and E=262144 edges (avg degree 64), the gather/segment-sum
message passing is densified: the normalized adjacency
A[d, s] = sum_{edges (s,d)} dinv[s]*dinv[d]  (+ dinv[i]^2 self loops)
is built on the host as a dense 4096x4096 matrix.  The whole network is then

    x1  = sigmoid(A @ (x @ W1) + b1)
    out = sigmoid(A @ (x1 @ W2) + b2)

i.e. two dense 4096^3 GEMMs plus small epilogues -- ideal for the PE array.

Sharding over 8 cores: layer-1 is column-sharded (each core computes 512
columns of H = x@W1 and of x1), the tiny layer-2 contraction
h2 = x1 @ W2 is computed per-core on the local 512 columns and reduced
across cores via 3 chunked bf16 AllGathers, each triggered as soon as its
m-tiles finish in GEMM2.  Platform collectives vary ~6x run-to-run and
serialize on one cc stream, so chunk scheduling, not bandwidth, is what
matters: early chunks' gathers + tree-sums are hoisted into the m-loop at
iterations where their waits are already met, the first 8 A-panels
prefetch during GEMM1 (16-deep pool rides out collective-induced DMA
stalls), and only the last ~2KB chunk's latency is exposed.  The final
out rows are row-sharded (512 per core); the final GEMM's k-slices run
per chunk as each h2 piece lands.

The two big GEMMs run in fp8-e4m3 with DoubleRow perf mode (2 contraction
rows per PE cell per cycle).  W1 and A are pre-scaled by 64 on the host so
their entries sit in e4m3's normal range; the scale is removed for free in
the activation epilogues (ACT computes func(in*scale + bias)).  PSUM
accumulation is fp32 throughout; the small final GEMM stays bf16.

W1 (2MB) is loaded across 4 DMA queues in parallel so the first GEMM-1
matmul can start at ~3us instead of ~14us.
"""

import numpy as np
import ml_dtypes

N = 4096
E = 262144
P = 128
NCORES = 8
JC = N // NCORES          # 512 feature-cols (layer1) / out-rows (final) per core
KT = N // P               # 32 contraction tiles
MT = N // P               # 32 output row tiles
WSCALE = 64.0             # host pre-scale on W1 and A for fp8 range

# h2-reduction collective chunks (m-tile ranges): few enough that
# slow-mode per-collective latency (~15-25us) stays off the critical
# path, triggered early, with the cc stream clear before the last
# chunk's data is ready.
CHUNKS = [(0, 12), (12, 24), (24, 32)]

_BF16 = ml_dtypes.bfloat16
_FP8 = ml_dtypes.float8_e4m3

_CACHE = {}


def _build_bass_program(with_b1=True):
    """Build + compile the SPMD Bass program (identical on all 8 cores)."""
    import concourse.bass as bass  # noqa: F401
    import concourse.bacc as bacc
    import concourse.tile as tile
    import concourse.mybir as mybir
    from concourse.bass_interp import get_hw_module

    dt = mybir.dt
    AF = mybir.ActivationFunctionType
    DR = mybir.MatmulPerfMode.DoubleRow

    nc = bacc.Bacc("TRN2", target_bir_lowering=False, debug=False,
                   num_devices=NCORES)

    # ---- kernel I/O (per-core) ----
    # xp_t[m, p, t*128+c] = x[m*128+c, t*128+p]      (pre-tiled lhsT panels)
    xp_t = nc.dram_tensor("xp_t", [MT, P, N], dt.float8e4, kind="ExternalInput")
    # ap_t[m, p, t*128+c] = 64*AT[t*128+p, m*128+c]  (AT[s,d] = A[d,s])
    ap_t = nc.dram_tensor("ap_t", [MT, P, N], dt.float8e4, kind="ExternalInput")
    # w1_t[p, t*512+j] = 64*W1[t*128+p, c*JC+j]
    w1_t = nc.dram_tensor("w1_t", [P, KT * JC], dt.float8e4, kind="ExternalInput")
    # atrc_t[p, t*512+j] = AT[t*128+p, c*JC+j]       (final-stage rhs panels)
    atrc_t = nc.dram_tensor("atrc_t", [P, KT * JC], dt.bfloat16, kind="ExternalInput")
    b1c = (nc.dram_tensor("b1c", [1, JC], dt.bfloat16, kind="ExternalInput")
           if with_b1 else None)  # 64*b1
    w2b = nc.dram_tensor("w2b", [P, JC], dt.float32, kind="ExternalInput")
    b2v = nc.dram_tensor("b2v", [1, 1], dt.float32, kind="ExternalInput")
    outc = nc.dram_tensor("outc", [1, JC], dt.float32, kind="ExternalOutput")

    with tile.TileContext(nc) as tc:
        with tc.tile_pool(name="const", bufs=1) as const, \
             tc.tile_pool(name="xpool", bufs=6) as xpool, \
             tc.tile_pool(name="apool", bufs=16) as apool, \
             tc.tile_pool(name="x1pool", bufs=3) as x1pool, \
             tc.tile_pool(name="pspool", bufs=2, space="PSUM") as pspool, \
             tc.tile_pool(name="psfinal", bufs=1, space="PSUM") as psfinal, \
             tc.tile_pool(name="drampool", bufs=1, space="DRAM") as drampool:

            # ---- resident SBUF tensors ----
            w1_sb = const.tile([P, KT, JC], dt.float8e4)
            atrc_sb = const.tile([P, KT * JC], dt.bfloat16)
            h_sb = const.tile([P, MT, JC], dt.float8e4)
            w2_sb = const.tile([P, JC], dt.float32)
            b1_sb = const.tile([1, JC], dt.bfloat16) if with_b1 else None
            b2_sb = const.tile([1, 1], dt.float32)
            ones_sb = const.tile([1, P], dt.bfloat16) if with_b1 else None
            p2_sb = const.tile([P, MT], dt.bfloat16)
            h2b_sb = const.tile([P, MT], dt.bfloat16)
            g_sb = [const.tile([P, NCORES, hi - lo], dt.bfloat16,
                                name=f"g_sb{i}")
                    for i, (lo, hi) in enumerate(CHUNKS)]
            o_sb = const.tile([1, JC], dt.float32)
            tts_sb = const.tile([P, JC], dt.float32)

            # ---- head: first matmul needs w1[0:2] + xp0[0:2]; spread the
            # 2MB w1 load over 4 DMA queues so GEMM1 is never weight-starved.
            xp0 = xpool.tile([P, KT, P], dt.float8e4, tag="xp")
            def w1_load(eng, k, ke):
                eng.dma_start(
                    w1_sb[:, k:ke, :],
                    w1_t.ap()[:, k * JC:ke * JC].rearrange(
                        "p (t j) -> p t j", j=JC))

            # k-interleaved across the scalar + gpsimd rings (balanced 1MB
            # each); m=0's k-loop is HBM-paced, so arrival order must be
            # k-ascending across both rings.
            w1_load(nc.scalar, 0, 2)
            nc.sync.dma_start(
                xp0[:, 0:4, :],
                xp_t.ap()[0].rearrange("p (t c) -> p t c", c=P)[:, 0:4, :])
            w1_load(nc.gpsimd, 2, 6)
            w1_load(nc.scalar, 6, 10)
            nc.sync.dma_start(
                xp0[:, 4:KT, :],
                xp_t.ap()[0].rearrange("p (t c) -> p t c", c=P)[:, 4:KT, :])
            w1_load(nc.gpsimd, 10, 14)
            w1_load(nc.scalar, 14, 18)
            w1_load(nc.gpsimd, 18, 22)
            w1_load(nc.scalar, 22, 26)
            w1_load(nc.gpsimd, 26, 30)
            w1_load(nc.scalar, 30, KT)
            nc.gpsimd.dma_start(w2_sb[:], w2b.ap())
            if with_b1:
                nc.gpsimd.dma_start(b1_sb[:], b1c.ap())
            nc.gpsimd.dma_start(b2_sb[:], b2v.ap())
            if with_b1:
                nc.vector.memset(ones_sb[:], 1.0)

            # Prefetch the first 8 A-panels on the scalar ring during GEMM1
            # (idle after the w1 head): GEMM2's start is then DMA-independent,
            # and the 16-deep apool rides out collective-induced DMA stalls.
            APRE = 8
            app_pre = []
            for m in range(APRE):
                app = apool.tile([P, KT, P], dt.float8e4, tag="app")
                nc.scalar.dma_start(
                    app[:], ap_t.ap()[m].rearrange("p (t c) -> p t c", c=P))
                app_pre.append(app)


            # ---- GEMM 1 (fp8 DoubleRow): 64*H[:, Cc] = x @ (64*W1[:, Cc]) ----
            for m in range(MT):
                if m == 0:
                    xp = xp0
                else:
                    xp = xpool.tile([P, KT, P], dt.float8e4, tag="xp")
                    nc.sync.dma_start(
                        xp[:], xp_t.ap()[m].rearrange("p (t c) -> p t c", c=P))
                ps1 = pspool.tile([P, JC], dt.float32, tag="ps1")
                for k in range(0, KT, 2):
                    nc.tensor.matmul(
                        ps1[:],
                        xp[:, k:k + 2, :],
                        w1_sb[:, k:k + 2, :],
                        start=(k == 0),
                        stop=(k == KT - 2),
                        perf_mode=DR,
                    )
                # PSUM -> SBUF: H = (64H)/64, cast to fp8 (rhs of GEMM 2)
                nc.scalar.mul(h_sb[:, m, :], ps1[:], 1.0 / WSCALE)

            # ---- GEMM 2 (fp8 DoubleRow): 64*O1 = (64*A) @ H + 64*b1 ;
            #      x1 = sigmoid(O1) ; partial2[i] = sum_j x1[i,j] * W2[Cc_j];
            #      per-chunk AllGather of partial2 as its m-tiles finish ----
            p2c_dram = [drampool.tile([P, hi - lo], dt.bfloat16,
                                       name=f"p2c{i}")
                        for i, (lo, hi) in enumerate(CHUNKS)]
            gc_dram = [drampool.tile([NCORES * P, hi - lo], dt.bfloat16,
                                     addr_space="Shared", name=f"gc{i}")
                       for i, (lo, hi) in enumerate(CHUNKS)]
            chunk_end = {hi - 1: i for i, (lo, hi) in enumerate(CHUNKS)}

            ATCH = (KT * JC) // 4
            for m in range(MT):
                if m < APRE:
                    app = app_pre[m]
                else:
                    app = apool.tile([P, KT, P], dt.float8e4, tag="app")
                    nc.sync.dma_start(
                        app[:], ap_t.ap()[m].rearrange("p (t c) -> p t c", c=P))
                if 1 <= m <= 4:
                    # final-stage rhs: 4 x ~1MB chunks slipped between the
                    # early ap panels (small, absorbable bubbles on the queue)
                    a0 = (m - 1) * ATCH
                    nc.sync.dma_start(atrc_sb[:, a0:a0 + ATCH],
                                      atrc_t.ap()[:, a0:a0 + ATCH])
                ps2 = pspool.tile([P, JC], dt.float32, tag="ps2", bufs=4)
                for k in range(0, KT, 2):
                    nc.tensor.matmul(
                        ps2[:],
                        app[:, k:k + 2, :],
                        h_sb[:, k:k + 2, :],
                        start=(k == 0),
                        stop=(not with_b1 and k == KT - 2),
                        perf_mode=DR,
                    )
                if with_b1:
                    # + 64*b1 as a bf16 rank-1 update: ones.T @ (64*b1)
                    nc.tensor.matmul(ps2[:], ones_sb[:], b1_sb[:],
                                     start=False, stop=True)
                x1t = x1pool.tile([P, JC], dt.float32, tag="x1t")
                # sigmoid((64*O1 + 64*b1) / 64)
                nc.scalar.activation(x1t[:], ps2[:], AF.Sigmoid, scale=1.0 / WSCALE)
                nc.vector.tensor_tensor(out=tts_sb[:], in0=x1t[:], in1=w2_sb[:],
                                        op=mybir.AluOpType.mult)
                with nc.allow_low_precision(reason="bf16 h2 partials on the wire; ~0.4% rel, budget 2e-2"):
                    nc.vector.tensor_reduce(out=p2_sb[:, m:m + 1], in_=tts_sb[:],
                                            axis=mybir.AxisListType.X,
                                            op=mybir.AluOpType.add)
                # hoisted gathers + tree-sums for the early chunks: issued
                # at m-iterations where their collective is already done (even
                # in slow mode), so the finals can start the moment GEMM2's
                # last matmul retires instead of after all 32 reduces.
                if m in (20, 30):
                    hci = 0 if m == 20 else 1
                    glo, ghi = CHUNKS[hci]
                    gview = gc_dram[hci].rearrange("(r p) m -> p r m", p=P)
                    nc.scalar.dma_start(g_sb[hci][:, 0:4, :], gview[:, 0:4, :])
                    nc.scalar.dma_start(g_sb[hci][:, 4:8, :], gview[:, 4:8, :])
                if m in (21, 31):
                    hci = 0 if m == 21 else 1
                    glo, ghi = CHUNKS[hci]
                    with nc.allow_low_precision(reason="bf16 h2 tree sum"):
                        nc.vector.tensor_tensor(
                            out=g_sb[hci][:, 0:4, :], in0=g_sb[hci][:, 0:4, :],
                            in1=g_sb[hci][:, 4:8, :], op=mybir.AluOpType.add)
                        nc.vector.tensor_tensor(
                            out=g_sb[hci][:, 0:2, :], in0=g_sb[hci][:, 0:2, :],
                            in1=g_sb[hci][:, 2:4, :], op=mybir.AluOpType.add)
                        nc.vector.tensor_tensor(
                            out=h2b_sb[:, glo:ghi], in0=g_sb[hci][:, 0, :],
                            in1=g_sb[hci][:, 1, :], op=mybir.AluOpType.add)
                ci = chunk_end.get(m)
                if ci is not None:
                    lo, hi = CHUNKS[ci]
                    # p2 chunk -> DRAM on the scalar queue (right after this
                    # chunk's sigmoid); trigger on gpsimd (collective_compute
                    # is gpsimd-only -- NRT straight-line order).  The last
                    # chunk's DMA goes inline on gpsimd instead: same-queue
                    # DMA+trigger skips a cross-engine semaphore hop on the
                    # critical path (safe here -- nothing later on the queue
                    # can cross-contaminate its wait thresholds).
                    dma_eng = nc.gpsimd if ci == len(CHUNKS) - 1 else nc.scalar
                    dma_eng.dma_start(p2c_dram[ci][:], p2_sb[:, lo:hi])
                    nc.gpsimd.collective_compute(
                        "AllGather", mybir.AluOpType.bypass,
                        replica_groups=[list(range(NCORES))],
                        ins=[p2c_dram[ci].opt()], outs=[gc_dram[ci].opt()])

            # ---- per chunk: gather -> 8-way tree sum -> final GEMM slice.
            # Chunk i's work only depends on collective i, so early chunks'
            # final matmuls overlap the last chunks' collectives.
            ps3 = psfinal.tile([1, JC], dt.float32, tag="ps3")
            last = len(CHUNKS) - 1
            lo, hi = CHUNKS[last]
            # last chunk: gather split across two queues (descriptor-count
            # bound), tree-sum, then its finals; earlier chunks' h2b pieces
            # were produced inside the m-loop.
            gview = gc_dram[last].rearrange("(r p) m -> p r m", p=P)
            nc.sync.dma_start(g_sb[last][:, 0:4, :], gview[:, 0:4, :])
            nc.scalar.dma_start(g_sb[last][:, 4:8, :], gview[:, 4:8, :])
            with nc.allow_low_precision(reason="bf16 8-way h2 tree sum; ~0.7% worst, budget 2e-2"):
                nc.vector.tensor_tensor(
                    out=g_sb[last][:, 0:4, :], in0=g_sb[last][:, 0:4, :],
                    in1=g_sb[last][:, 4:8, :], op=mybir.AluOpType.add)
                nc.vector.tensor_tensor(
                    out=g_sb[last][:, 0:2, :], in0=g_sb[last][:, 0:2, :],
                    in1=g_sb[last][:, 2:4, :], op=mybir.AluOpType.add)
                nc.vector.tensor_tensor(
                    out=h2b_sb[:, lo:hi], in0=g_sb[last][:, 0, :],
                    in1=g_sb[last][:, 1, :], op=mybir.AluOpType.add)
            for k in range(KT):
                nc.tensor.matmul(
                    ps3[:],
                    h2b_sb[:, k:k + 1],
                    atrc_sb[:, k * JC:(k + 1) * JC],
                    start=(k == 0),
                    stop=(k == KT - 1),
                )

            # ---- out[Rc]^T = sigmoid(h2^T @ AT[:, Rc] + b2) ----
            # out DMA on the scalar queue, right behind the ACT (no
            # cross-engine semaphore hop).
            nc.scalar.activation(o_sb[:], ps3[:], AF.Sigmoid, bias=b2_sb[:])
            nc.scalar.dma_start(outc.ap(), o_sb[:])

    nc.compile()
    nc.m = get_hw_module(nc.m)
    return nc


def _host_preprocess(x, edge_index, W1, b1, W2, b2):
    """Build dense AT + pre-tiled fp8/bf16 operands; returns per-core in_maps."""
    edge_index = np.asarray(edge_index)
    src = edge_index[0].astype(np.int64)
    dst = edge_index[1].astype(np.int64)
    deg = np.bincount(dst, minlength=N).astype(np.float64) + 1.0
    dinv = 1.0 / np.sqrt(deg)
    vals = dinv[src] * dinv[dst]
    # AT[s, d] = A[d, s] (accumulates duplicate edges, like segment_sum)
    AT = np.bincount(src * N + dst, weights=vals, minlength=N * N)
    AT = AT.reshape(N, N)
    idx = np.arange(N)
    AT[idx, idx] += dinv * dinv
    AT32 = AT.astype(np.float32)

    x32 = np.asarray(x, dtype=np.float32)
    W1_32 = np.asarray(W1, dtype=np.float32)
    b1_32 = np.asarray(b1, dtype=np.float32)
    W2_32 = np.asarray(W2, dtype=np.float32).reshape(N)
    b2_32 = np.asarray(b2, dtype=np.float32).reshape(1)

    # xp_t[m, p, t*128+c] = x[m*128+c, t*128+p]
    xp_t = np.ascontiguousarray(
        x32.reshape(MT, P, KT, P).transpose(0, 3, 2, 1).reshape(MT, P, N)
    ).astype(_FP8)
    # ap_t[m, p, t*128+c] = 64*AT[t*128+p, m*128+c]
    ap_t = np.ascontiguousarray(
        (AT32 * np.float32(WSCALE)).reshape(KT, P, MT, P)
        .transpose(2, 1, 0, 3).reshape(MT, P, N)
    ).astype(_FP8)

    AT_b = AT32.astype(_BF16)
    W1_s = (W1_32 * np.float32(WSCALE)).astype(_FP8)

    in_maps = []
    for c in range(NCORES):
        cols = slice(c * JC, (c + 1) * JC)
        w1_t = np.ascontiguousarray(
            W1_s[:, cols].reshape(KT, P, JC).transpose(1, 0, 2).reshape(P, KT * JC)
        )
        atrc_t = np.ascontiguousarray(
            AT_b[:, cols].reshape(KT, P, JC).transpose(1, 0, 2).reshape(P, KT * JC)
        )
        in_maps.append({
            "xp_t": xp_t,
            "ap_t": ap_t,
            "w1_t": w1_t,
            "atrc_t": atrc_t,
            "b1c": (b1_32[cols] * np.float32(WSCALE)).reshape(1, JC).astype(_BF16),
            "w2b": np.ascontiguousarray(
                np.broadcast_to(W2_32[cols][None, :], (P, JC))
            ).astype(np.float32),
            "b2v": b2_32.reshape(1, 1).astype(np.float32),
        })
    return in_maps


def kernel(x, edge_index, W1, b1, W2, b2, _trace=False, _premaps=None):
    from concourse import bass_utils

    with_b1 = bool(np.any(np.asarray(b1)))
    key = f"nc_b1={with_b1}"
    if key not in _CACHE:
        _CACHE[key] = _build_bass_program(with_b1=with_b1)
    nc = _CACHE[key]

    in_maps = _premaps if _premaps is not None else _host_preprocess(
        x, edge_index, W1, b1, W2, b2)
    if not with_b1:
        in_maps = [{k: v for k, v in m.items() if k != "b1c"} for m in in_maps]

    res = bass_utils.run_bass_kernel_spmd(
        nc, in_maps, core_ids=list(range(NCORES)), trace=_trace,
    )
    out = np.concatenate(
        [np.asarray(res.results[c]["outc"]).reshape(JC) for c in range(NCORES)]
    ).reshape(N, 1).astype(np.float32)
    if _trace:
        _CACHE["last_result"] = res
    return out

